# revision 6
# baseline (speedup 1.0000x reference)
"""CrossAttn2d Trainium2 kernel — data-parallel over batch (B=8 -> 8 cores).

Math (per batch b, all hardcoded for B=8, CQ=128, CKV=64, H=W=64, DIM=16):
  q = 0.25*(Wq @ x4q + bq)          [16, 4096]   (0.25 = dim^-0.5 folded in)
  k = Wk @ x4kv + bk                [16, 4096]
  v0 = Wv @ x4kv                    [128, 4096]  (bias bv pulled out, see below)
  scoreT[m,n] = sum_d k[d,m] q[d,n]             (transposed score: m on partitions)
  e = exp(scoreT)                   (no max-subtract: |score| <~ 1.5)
  den[n] = sum_m e[m,n]             (ones-matmul, broadcast over partitions)
  VW = (gamma*Wo) @ v0              (out-projection folded into v ahead of time)
  Y[c,n] = sum_m VW[c,m] e[m,n]
  out = Y/den + (gamma*(Wo@bv + bo) + x4q)
"""

import os
import numpy as np
import ml_dtypes

BF16 = ml_dtypes.bfloat16

_PROGRAM = None


def _build_program():
    import concourse.bass as bass
    import concourse.mybir as mybir
    import concourse.tile as tile
    from concourse import bacc

    f32 = mybir.dt.float32
    bf16 = mybir.dt.bfloat16
    AF = mybir.ActivationFunctionType

    nc = bacc.Bacc(None)

    xq_d = nc.dram_tensor("xq", [128, 4096], f32, kind="ExternalInput")
    xkv_d = nc.dram_tensor("xkv", [64, 4096], f32, kind="ExternalInput")
    wqT_d = nc.dram_tensor("wqT", [128, 16], bf16, kind="ExternalInput")
    wkT_d = nc.dram_tensor("wkT", [64, 16], bf16, kind="ExternalInput")
    wvT_d = nc.dram_tensor("wvT", [64, 128], bf16, kind="ExternalInput")
    woT_d = nc.dram_tensor("woT", [128, 128], bf16, kind="ExternalInput")
    bq_d = nc.dram_tensor("bq", [16, 1], f32, kind="ExternalInput")
    bk_d = nc.dram_tensor("bk", [16, 1], f32, kind="ExternalInput")
    bog_d = nc.dram_tensor("bog", [128, 1], f32, kind="ExternalInput")
    ones_d = nc.dram_tensor("ones", [128, 128], bf16, kind="ExternalInput")
    out_d = nc.dram_tensor("out", [128, 4096], f32, kind="ExternalOutput")

    with tile.TileContext(nc) as tc:
        with (
            tc.tile_pool(name="const", bufs=1) as const,
            tc.tile_pool(name="big", bufs=1) as big,
            tc.tile_pool(name="expp", bufs=3) as expp,
            tc.tile_pool(name="eps", bufs=2) as eps,
        ):
            # ---- load inputs + weights ----
            xq_sb = big.tile([128, 4096], f32, tag="xq_sb")
            nc.sync.dma_start(xq_sb[:], xq_d[:])
            xkv_sb = big.tile([64, 4096], f32, tag="xkv_sb")
            nc.sync.dma_start(xkv_sb[:], xkv_d[:])
            wqT = const.tile([128, 16], bf16, tag="wqT")
            nc.sync.dma_start(wqT[:], wqT_d[:])
            wkT = const.tile([64, 16], bf16, tag="wkT")
            nc.sync.dma_start(wkT[:], wkT_d[:])
            wvT = const.tile([64, 128], bf16, tag="wvT")
            nc.sync.dma_start(wvT[:], wvT_d[:])
            woT = const.tile([128, 128], bf16, tag="woT")
            nc.sync.dma_start(woT[:], woT_d[:])
            bq_sb = const.tile([16, 1], f32, tag="bq_sb")
            nc.sync.dma_start(bq_sb[:], bq_d[:])
            bk_sb = const.tile([16, 1], f32, tag="bk_sb")
            nc.sync.dma_start(bk_sb[:], bk_d[:])
            bog_sb = const.tile([128, 1], f32, tag="bog_sb")
            nc.sync.dma_start(bog_sb[:], bog_d[:])
            ones_sb = const.tile([128, 128], bf16, tag="ones_sb")
            nc.sync.dma_start(ones_sb[:], ones_d[:])

            # ---- bf16 casts of activations ----
            xq_bf = big.tile([128, 4096], bf16, tag="xq_bf")
            nc.vector.tensor_copy(xq_bf[:], xq_sb[:])
            xkv_bf = big.tile([64, 4096], bf16, tag="xkv_bf")
            nc.vector.tensor_copy(xkv_bf[:], xkv_sb[:])

            # residual + folded bias, precomputed once: xq_plus = xq + bog
            xq_plus = big.tile([128, 4096], f32, tag="xq_plus")
            nc.scalar.activation(xq_plus[:], xq_sb[:], AF.Identity, bias=bog_sb[:])

            Qb = big.tile([16, 4096], bf16, tag="Qb")
            Kb = big.tile([16, 4096], bf16, tag="Kb")
            V_sb = big.tile([128, 4096], bf16, tag="V_sb")
            VWT = big.tile([128, 4096], bf16, tag="VWT")

            # ---- projections ----
            with tc.tile_pool(name="ps_proj", bufs=2, space="PSUM") as ps_proj:
                for j in range(8):
                    js = slice(512 * j, 512 * (j + 1))
                    # q = 0.25*(Wq x + bq): lhsT=wqT [128,16], rhs=xq chunk
                    pq = ps_proj.tile([128, 512], f32, tag="ps")
                    nc.tensor.matmul(pq[0:16, :], wqT[:], xq_bf[:, js],
                                     start=True, stop=True)
                    nc.scalar.activation(Qb[:, js], pq[0:16, :], AF.Identity,
                                         bias=bq_sb[:])
                for j in range(8):
                    js = slice(512 * j, 512 * (j + 1))
                    pk = ps_proj.tile([128, 512], f32, tag="ps")
                    nc.tensor.matmul(pk[0:16, :], wkT[:], xkv_bf[:, js],
                                     start=True, stop=True)
                    nc.scalar.activation(Kb[:, js], pk[0:16, :], AF.Identity,
                                         bias=bk_sb[:])
                for j in range(8):
                    js = slice(512 * j, 512 * (j + 1))
                    pv = ps_proj.tile([128, 512], f32, tag="ps")
                    nc.tensor.matmul(pv[:], wvT[:], xkv_bf[:, js],
                                     start=True, stop=True)
                    nc.vector.tensor_copy(V_sb[:, js], pv[:])
                # VWT[m, c2] per 128-m-tile: lhsT=V_sb[:,t] [128c,128m], rhs=woT
                for g in range(8):
                    pw = ps_proj.tile([128, 512], f32, tag="ps")
                    for u in range(4):
                        t = 4 * g + u
                        nc.tensor.matmul(
                            pw[:, 128 * u:128 * (u + 1)],
                            V_sb[:, 128 * t:128 * (t + 1)], woT[:],
                            start=True, stop=True)
                    nc.vector.tensor_copy(VWT[:, 512 * g:512 * (g + 1)], pw[:])

            # ---- main attention loop ----
            with (
                tc.tile_pool(name="ps_s", bufs=2, space="PSUM") as ps_s,
                tc.tile_pool(name="ps_av", bufs=2, space="PSUM") as ps_av,
                tc.tile_pool(name="ps_den", bufs=2, space="PSUM") as ps_den,
            ):
                for j in range(8):
                    js = slice(512 * j, 512 * (j + 1))
                    av = ps_av.tile([128, 512], f32, tag="av")
                    den = ps_den.tile([128, 512], f32, tag="den")
                    for g in range(16):
                        ps = ps_s.tile([128, 1024], f32, tag="s")
                        et = expp.tile([128, 1024], bf16, tag="et")
                        for u in range(2):
                            t = 2 * g + u
                            nc.tensor.matmul(
                                ps[:, 512 * u:512 * (u + 1)],
                                Kb[:, 128 * t:128 * (t + 1)], Qb[:, js],
                                start=True, stop=True)
                        nc.scalar.activation(et[:], ps[:], AF.Exp)
                        for u in range(2):
                            t = 2 * g + u
                            first = (g == 0 and u == 0)
                            last = (g == 15 and u == 1)
                            es = et[:, 512 * u:512 * (u + 1)]
                            nc.tensor.matmul(
                                av[:], VWT[:, 128 * t:128 * (t + 1)], es,
                                start=first, stop=last)
                            nc.tensor.matmul(
                                den[:], ones_sb[:], es,
                                start=first, stop=last)
                    # epilogue: out = av/den + xq_plus
                    rec = eps.tile([128, 512], f32, tag="rec")
                    nc.vector.reciprocal(rec[:], den[:])
                    y1 = eps.tile([128, 512], f32, tag="y1")
                    nc.vector.tensor_mul(y1[:], av[:], rec[:])
                    ot = eps.tile([128, 512], f32, tag="ot")
                    nc.vector.tensor_add(ot[:], y1[:], xq_plus[:, js])
                    nc.sync.dma_start(out_d[:, js], ot[:])

    nc.finalize()
    return nc


def _get_program():
    global _PROGRAM
    if _PROGRAM is None:
        _PROGRAM = _build_program()
    return _PROGRAM


def _make_in_maps(inputs):
    return _make_in_maps_args(**inputs)


def _make_in_maps_args(x4q, x4kv, Wq, bq, Wk, bk, Wv, bv, Wo, bo, gamma):
    B = 8
    g = np.float32(gamma.reshape(-1)[0])
    wqT = np.ascontiguousarray((Wq * 0.25).T).astype(BF16)        # [128, 16]
    bq_ = (bq * 0.25).astype(np.float32).reshape(16, 1)
    wkT = np.ascontiguousarray(Wk.T).astype(BF16)                 # [64, 16]
    bk_ = bk.astype(np.float32).reshape(16, 1)
    wvT = np.ascontiguousarray(Wv.T).astype(BF16)                 # [64, 128]
    woT = np.ascontiguousarray((g * Wo).T).astype(BF16)           # [128, 128]
    bog = (g * (Wo.astype(np.float64) @ bv.astype(np.float64)
                + bo.astype(np.float64))).astype(np.float32).reshape(128, 1)
    ones = np.ones((128, 128), BF16)

    shared = dict(wqT=wqT, wkT=wkT, wvT=wvT, woT=woT,
                  bq=bq_, bk=bk_, bog=bog, ones=ones)
    in_maps = [
        dict(xq=np.ascontiguousarray(x4q[i].reshape(128, 4096)),
             xkv=np.ascontiguousarray(x4kv[i].reshape(64, 4096)),
             **shared)
        for i in range(B)
    ]
    return in_maps


def kernel(x4q, x4kv, Wq, bq, Wk, bk, Wv, bv, Wo, bo, gamma):
    from concourse.bass_utils import run_bass_kernel_spmd

    in_maps = _make_in_maps_args(x4q, x4kv, Wq, bq, Wk, bk, Wv, bv,
                                 Wo, bo, gamma)
    nc = _get_program()
    res = run_bass_kernel_spmd(nc, in_maps, list(range(8)))
    out = np.stack([np.asarray(res.results[i]["out"], dtype=np.float32)
                    .reshape(128, 64, 64) for i in range(8)])
    return out


# revision 12
# speedup vs baseline: 1.1598x; 1.1598x over previous
"""CrossAttn2d Trainium2 kernel — data-parallel over batch (B=8 -> 8 cores).

Math (per batch b, all hardcoded for B=8, CQ=128, CKV=64, H=W=64, DIM=16):
  q = 0.25*(Wq @ x4q + bq)          [16, 4096]   (0.25 = dim^-0.5 folded in)
  k = Wk @ x4kv + bk                [16, 4096]
  v0 = Wv @ x4kv                    [128, 4096]  (bias bv pulled out, see below)
  scoreT[m,n] = sum_d k[d,m] q[d,n]             (transposed score: m on partitions)
  e = exp(scoreT)                   (no max-subtract: |score| <~ 1.5)
  den[n] = sum_m e[m,n]             (ones-matmul, broadcast over partitions)
  VW = (gamma*Wo) @ v0              (out-projection folded into v ahead of time)
  Y[c,n] = sum_m VW[c,m] e[m,n]
  out = Y/den + (gamma*(Wo@bv + bo) + x4q)
"""

import os
import numpy as np
import ml_dtypes

BF16 = ml_dtypes.bfloat16

_PROGRAM = None


def _build_program():
    import concourse.bass as bass
    import concourse.mybir as mybir
    import concourse.tile as tile
    from concourse import bacc

    f32 = mybir.dt.float32
    bf16 = mybir.dt.bfloat16
    AF = mybir.ActivationFunctionType

    nc = bacc.Bacc(None)

    xq_d = nc.dram_tensor("xq", [128, 4096], f32, kind="ExternalInput")
    xkv_d = nc.dram_tensor("xkv", [64, 4096], f32, kind="ExternalInput")
    wqT_d = nc.dram_tensor("wqT", [128, 16], bf16, kind="ExternalInput")
    wkT_d = nc.dram_tensor("wkT", [64, 16], bf16, kind="ExternalInput")
    wvT_d = nc.dram_tensor("wvT", [64, 128], bf16, kind="ExternalInput")
    woT_d = nc.dram_tensor("woT", [128, 128], bf16, kind="ExternalInput")
    bq_d = nc.dram_tensor("bq", [128, 1], f32, kind="ExternalInput")
    bk_d = nc.dram_tensor("bk", [128, 1], f32, kind="ExternalInput")
    bog_d = nc.dram_tensor("bog", [128, 1], f32, kind="ExternalInput")
    ones_d = nc.dram_tensor("ones", [128, 128], bf16, kind="ExternalInput")
    out_d = nc.dram_tensor("out", [128, 4096], f32, kind="ExternalOutput")

    with tile.TileContext(nc) as tc:
        with (
            tc.tile_pool(name="const", bufs=1) as const,
            tc.tile_pool(name="big", bufs=1) as big,
            tc.tile_pool(name="expp", bufs=3) as expp,
            tc.tile_pool(name="eps", bufs=2) as eps,
        ):
            # ---- load inputs + weights ----
            xq_sb = big.tile([128, 4096], f32, tag="xq_sb")
            nc.sync.dma_start(xq_sb[:], xq_d[:])
            xkv_sb = big.tile([64, 4096], f32, tag="xkv_sb")
            nc.sync.dma_start(xkv_sb[:], xkv_d[:])
            wqT = const.tile([128, 16], bf16, tag="wqT")
            nc.sync.dma_start(wqT[:], wqT_d[:])
            wkT = const.tile([64, 16], bf16, tag="wkT")
            nc.sync.dma_start(wkT[:], wkT_d[:])
            wvT = const.tile([64, 128], bf16, tag="wvT")
            nc.sync.dma_start(wvT[:], wvT_d[:])
            woT = const.tile([128, 128], bf16, tag="woT")
            nc.sync.dma_start(woT[:], woT_d[:])
            bq_sb = const.tile([128, 1], f32, tag="bq_sb")
            nc.sync.dma_start(bq_sb[:], bq_d[:])
            bk_sb = const.tile([128, 1], f32, tag="bk_sb")
            nc.sync.dma_start(bk_sb[:], bk_d[:])
            bog_sb = const.tile([128, 1], f32, tag="bog_sb")
            nc.sync.dma_start(bog_sb[:], bog_d[:])
            ones_sb = const.tile([128, 128], bf16, tag="ones_sb")
            nc.sync.dma_start(ones_sb[:], ones_d[:])

            # ---- bf16 casts of activations ----
            xq_bf = big.tile([128, 4096], bf16, tag="xq_bf")
            nc.vector.tensor_copy(xq_bf[:], xq_sb[:])
            xkv_bf = big.tile([64, 4096], bf16, tag="xkv_bf")
            nc.vector.tensor_copy(xkv_bf[:], xkv_sb[:])

            # residual + folded bias, precomputed once: xq_plus = xq + bog
            xq_plus = big.tile([128, 4096], f32, tag="xq_plus")
            nc.scalar.activation(xq_plus[:], xq_sb[:], AF.Identity, bias=bog_sb[:])

            # Q/K replicated at partition groups {0-15, 32-47} so score
            # matmuls can be row-packed 2x via tile_position.
            Qb = big.tile([64, 4096], bf16, tag="Qb")
            Kb = big.tile([64, 4096], bf16, tag="Kb")
            V_sb = big.tile([128, 4096], bf16, tag="V_sb")
            VWT = big.tile([128, 4096], bf16, tag="VWT")

            # ---- projections ----
            with tc.tile_pool(name="ps_proj", bufs=2, space="PSUM") as ps_proj:
                for j in range(8):
                    js = slice(512 * j, 512 * (j + 1))
                    # q = 0.25*(Wq x + bq): lhsT=wqT [128,16], rhs=xq chunk
                    pq = ps_proj.tile([128, 512], f32, tag="ps")
                    nc.tensor.matmul(pq[0:16, :], wqT[:], xq_bf[:, js],
                                     start=True, stop=True)
                    nc.scalar.activation(Qb[0:16, js], pq[0:16, :], AF.Identity,
                                         bias=bq_sb[0:16])
                for j in range(8):
                    js = slice(512 * j, 512 * (j + 1))
                    pk = ps_proj.tile([128, 512], f32, tag="ps")
                    nc.tensor.matmul(pk[0:16, :], wkT[:], xkv_bf[:, js],
                                     start=True, stop=True)
                    nc.scalar.activation(Kb[0:16, js], pk[0:16, :], AF.Identity,
                                         bias=bk_sb[0:16])
                # replicate Q/K to partition group 32-47 for row-packed MMs
                nc.sync.dma_start(Qb[32:48, :], Qb[0:16, :])
                nc.sync.dma_start(Kb[32:48, :], Kb[0:16, :])
                for j in range(8):
                    js = slice(512 * j, 512 * (j + 1))
                    pv = ps_proj.tile([128, 512], f32, tag="ps")
                    nc.tensor.matmul(pv[:], wvT[:], xkv_bf[:, js],
                                     start=True, stop=True)
                    nc.vector.tensor_copy(V_sb[:, js], pv[:])
                # VWT[m, c2] per 128-m-tile: lhsT=V_sb[:,t] [128c,128m], rhs=woT
                for g in range(8):
                    pw = ps_proj.tile([128, 512], f32, tag="ps")
                    for u in range(4):
                        t = 4 * g + u
                        nc.tensor.matmul(
                            pw[:, 128 * u:128 * (u + 1)],
                            V_sb[:, 128 * t:128 * (t + 1)], woT[:],
                            start=True, stop=True)
                    nc.vector.tensor_copy(VWT[:, 512 * g:512 * (g + 1)], pw[:])

            # ---- main attention loop ----
            with (
                tc.tile_pool(name="ps_s", bufs=2, space="PSUM") as ps_s,
                tc.tile_pool(name="ps_av", bufs=2, space="PSUM") as ps_av,
                tc.tile_pool(name="ps_den", bufs=2, space="PSUM") as ps_den,
                tc.tile_pool(name="accp", bufs=2) as accp,
            ):
                for j in range(8):
                    js = slice(512 * j, 512 * (j + 1))
                    av = ps_av.tile([128, 512], f32, tag="av")
                    acc = accp.tile([128, 1024], bf16, tag="acc")
                    for g in range(16):
                        ps = ps_s.tile([128, 1024], f32, tag="s")
                        et = expp.tile([128, 1024], bf16, tag="et")
                        for u in range(2):
                            t = 2 * g + u
                            nc.tensor.matmul(
                                ps[:, 512 * u:512 * (u + 1)],
                                Kb[32 * u:32 * u + 16, 128 * t:128 * (t + 1)],
                                Qb[32 * u:32 * u + 16, js],
                                start=True, stop=True,
                                tile_position=(32 * u, 0))
                        nc.scalar.activation(et[:], ps[:], AF.Exp)
                        for u in range(2):
                            t = 2 * g + u
                            first = (g == 0 and u == 0)
                            last = (g == 15 and u == 1)
                            es = et[:, 512 * u:512 * (u + 1)]
                            nc.tensor.matmul(
                                av[:], VWT[:, 128 * t:128 * (t + 1)], es,
                                start=first, stop=last)
                        # denominator partials on DVE (bf16, 2x mode)
                        if g == 0:
                            nc.vector.tensor_copy(acc[:], et[:])
                        else:
                            nc.vector.tensor_add(acc[:], acc[:], et[:])
                    # fold the two 512-halves, then reduce over partitions
                    # with a ones-matmul (broadcasts denom to all partitions)
                    acc512 = accp.tile([128, 512], bf16, tag="acc512")
                    nc.vector.tensor_add(acc512[:], acc[:, 0:512],
                                         acc[:, 512:1024])
                    den = ps_den.tile([128, 512], f32, tag="den")
                    nc.tensor.matmul(den[:], ones_sb[:], acc512[:],
                                     start=True, stop=True)
                    # epilogue: out = av/den + xq_plus
                    rec = eps.tile([128, 512], f32, tag="rec")
                    nc.vector.reciprocal(rec[:], den[:])
                    y1 = eps.tile([128, 512], f32, tag="y1")
                    nc.vector.tensor_mul(y1[:], av[:], rec[:])
                    ot = eps.tile([128, 512], f32, tag="ot")
                    nc.vector.tensor_add(ot[:], y1[:], xq_plus[:, js])
                    nc.sync.dma_start(out_d[:, js], ot[:])

    nc.finalize()
    return nc


def _get_program():
    global _PROGRAM
    if _PROGRAM is None:
        _PROGRAM = _build_program()
    return _PROGRAM


def _make_in_maps(inputs):
    return _make_in_maps_args(**inputs)


def _make_in_maps_args(x4q, x4kv, Wq, bq, Wk, bk, Wv, bv, Wo, bo, gamma):
    B = 8
    g = np.float32(gamma.reshape(-1)[0])
    wqT = np.ascontiguousarray((Wq * 0.25).T).astype(BF16)        # [128, 16]
    bq_ = np.zeros((128, 1), np.float32)
    bq_[0:16, 0] = bq * 0.25
    bq_[32:48, 0] = bq * 0.25
    wkT = np.ascontiguousarray(Wk.T).astype(BF16)                 # [64, 16]
    bk_ = np.zeros((128, 1), np.float32)
    bk_[0:16, 0] = bk
    bk_[32:48, 0] = bk
    wvT = np.ascontiguousarray(Wv.T).astype(BF16)                 # [64, 128]
    woT = np.ascontiguousarray((g * Wo).T).astype(BF16)           # [128, 128]
    bog = (g * (Wo.astype(np.float64) @ bv.astype(np.float64)
                + bo.astype(np.float64))).astype(np.float32).reshape(128, 1)
    ones = np.ones((128, 128), BF16)

    shared = dict(wqT=wqT, wkT=wkT, wvT=wvT, woT=woT,
                  bq=bq_, bk=bk_, bog=bog, ones=ones)
    in_maps = [
        dict(xq=np.ascontiguousarray(x4q[i].reshape(128, 4096)),
             xkv=np.ascontiguousarray(x4kv[i].reshape(64, 4096)),
             **shared)
        for i in range(B)
    ]
    return in_maps


def kernel(x4q, x4kv, Wq, bq, Wk, bk, Wv, bv, Wo, bo, gamma):
    from concourse.bass_utils import run_bass_kernel_spmd

    in_maps = _make_in_maps_args(x4q, x4kv, Wq, bq, Wk, bk, Wv, bv,
                                 Wo, bo, gamma)
    nc = _get_program()
    res = run_bass_kernel_spmd(nc, in_maps, list(range(8)))
    out = np.stack([np.asarray(res.results[i]["out"], dtype=np.float32)
                    .reshape(128, 64, 64) for i in range(8)])
    return out
